# revision 38
# baseline (speedup 1.0000x reference)
"""Trainium2 Bass kernel for the MemoryEfficientVoxelizer problem.

Splats N=512 3D Gaussians onto an (80, 80, 12) voxel grid:
    contrib[n, v] = opac[n] * exp(-0.5 * (g_v - mu_n)^T Cinv_n (g_v - mu_n))
                    * [ |g_v - mu_n|^2 < (3 * sigma_max_n)^2 ]
    density[v]    = sum_n contrib[n, v]
    feats[v, :]   = (sum_n contrib[n, v] * features[n, :]) / max(density[v], 1e-6)

Strategy (8 NeuronCores, voxel-grid sharded):
  - Partition the grid into 4x8x12-voxel blocks (200 blocks, 384 voxels each).
  - On host: cull Gaussians per block (point-to-bbox distance vs the 3*sigma_max
    sphere) giving a sparse (block, gaussian) pair list; balance pairs across
    cores; bin-pack each core's blocks into 128-pair chunks.
  - Both the Mahalanobis form and the sphere test are quadratic polynomials in
    the voxel coordinates, so per chunk the device computes them as tiny
    matmuls against a *shared* block-local polynomial basis:
        maha_arg[pair, v] = W[pair, :10] @ B10[:, v]   (K=10, fp32)
        negd2c[pair, v]   = U[pair, :5]  @ B5[:, v]    (K=5,  fp32, row group 2)
    with -0.5 and log(opacity) folded into W on the host.
  - contrib = (negd2c > 0) * exp(maha_arg): one ScalarE Exp + one fused
    VectorE scalar_tensor_tensor.
  - Feature/density accumulation per 128-voxel slice: contrib slice (stationary)
    x zero-masked per-block feature columns (features + ones column), giving
    [voxel, 33] tiles; normalization is then per-partition native.
"""

import os
import numpy as np
import ml_dtypes
from contextlib import ExitStack

import concourse.bass as bass
import concourse.tile as tile
from concourse import bacc, mybir
from concourse import bass_utils

# ---- problem constants (fixed by the reference) ----
VOL_MIN = np.array([-20.0, -20.0, -2.0], np.float32)
VOL_MAX = np.array([20.0, 20.0, 4.4], np.float32)
VOX = 0.5
GS = (80, 80, 12)
F = 32
OPACITY_THRESH = 1e-4
N_CORES = 8

# block shape in voxels
BX, BY, BZ = 4, 8, 12
VB = BX * BY * BZ          # 384 voxels / block
NVC = VB // 128            # 3 voxel slices of 128
FCOL = F + 1               # features + ones column

LAST_RESULTS = None        # BassKernelResults of the last hardware run
_BUILD_CACHE = {}


def _grid_coords():
    coords = [np.arange(s, dtype=np.float32) + 0.5 for s in GS]
    g = np.stack(np.meshgrid(*coords, indexing="ij"), axis=-1)
    return g * VOX + VOL_MIN  # [80,80,12,3] voxel centers


def _prepare(means3d, opacities, covariances, features):
    """Host-side culling, packing and weight computation.

    Returns (cfg, in_maps, scatter) where scatter maps device output slots back
    to grid blocks.
    """
    g = _grid_coords()
    sigma2 = np.diagonal(covariances, axis1=1, axis2=2).astype(np.float64)
    sigma = np.sqrt(sigma2).astype(np.float32)
    op = opacities[:, 0].astype(np.float32).copy()
    keep = ((means3d + 3 * sigma) > VOL_MIN).all(1) & \
           ((means3d - 3 * sigma) < VOL_MAX).all(1) & (op > OPACITY_THRESH)
    op = op * keep
    cinv = np.linalg.inv(covariances.astype(np.float64))
    smax = sigma.max(1)
    R2 = ((3.0 * smax) ** 2).astype(np.float32)  # squared sphere radii

    nbx, nby, nbz = GS[0] // BX, GS[1] // BY, GS[2] // BZ
    blocks = []  # (bx, by, bz, center[3], sel_indices)
    live = np.where(op > 0)[0]
    for bx in range(nbx):
        for by in range(nby):
            for bz in range(nbz):
                vox = g[bx*BX:(bx+1)*BX, by*BY:(by+1)*BY, bz*BZ:(bz+1)*BZ]
                lo = vox.reshape(-1, 3).min(0)
                hi = vox.reshape(-1, 3).max(0)
                center = (lo + hi) * 0.5
                d = np.maximum(np.maximum(lo - means3d[live], means3d[live] - hi), 0.0)
                sel = live[(d * d).sum(1) < R2[live]]
                if len(sel):
                    blocks.append((bx, by, bz, center, sel))

    # greedy balance blocks across cores by pair count
    blocks.sort(key=lambda b: -len(b[4]))
    core_blocks = [[] for _ in range(N_CORES)]
    core_pairs = [0] * N_CORES
    for b in blocks:
        c = min(range(N_CORES), key=lambda i: (core_pairs[i], len(core_blocks[i])))
        core_blocks[c].append(b)
        core_pairs[c] += len(b[4])

    # choose (NCHUNK, NBC) and first-fit-decreasing bin packing per core
    def pack(blks, nchunk, nbc):
        bins = [[] for _ in range(nchunk)]
        cnt = [0] * nchunk
        for b in sorted(blks, key=lambda b: -len(b[4])):
            placed = False
            for i in range(nchunk):
                if cnt[i] + len(b[4]) <= 128 and len(bins[i]) < nbc:
                    bins[i].append(b)
                    cnt[i] += len(b[4])
                    placed = True
                    break
            if not placed:
                return None
        return bins

    packing = None
    for nchunk in range(1, 16):
        maxb = max((len(cb) for cb in core_blocks), default=1)
        for nbc in range(max(2, -(-maxb // nchunk)), 16):
            if nbc % 2:
                continue
            if (nbc // 2) * FCOL * NVC > 512:
                break
            trial = [pack(cb, nchunk, nbc) for cb in core_blocks]
            if all(t is not None for t in trial):
                packing = trial
                NCHUNK, NBC = nchunk, nbc
                break
        if packing is not None:
            break
    assert packing is not None, "bin packing failed (unexpected input distribution)"

    cfg = (NCHUNK, NBC)

    def split2(a):  # bf16 Kahan pair: a ~= hi + lo
        hi = a.astype(ml_dtypes.bfloat16).astype(np.float32)
        lo = (a - hi).astype(ml_dtypes.bfloat16).astype(np.float32)
        return hi, lo

    # Shared block-local polynomial basis. Quadratic forms run as single bf16
    # matmuls via a Kahan split: stat [Xh; Xh; Xl] against basis [Bh; Bl; Bh]
    # computes XhBh + XhBl + XlBh in one K-deep contraction (bf16 products are
    # exact in fp32).
    #   group 0 (-> partitions 0..29):  3x10 rows, Mahalanobis basis
    #   group 1 (-> partitions 64..78): 3x5 rows, sphere-test basis
    loc = g[:BX, :BY, :BZ].reshape(-1, 3)
    loc = loc - (loc.min(0) + loc.max(0)) * 0.5  # block-local coords (same for all blocks)
    x, y, z = loc[:, 0], loc[:, 1], loc[:, 2]
    one = np.ones_like(x)
    bm = np.stack([x*x, y*y, z*z, x*y, x*z, y*z, x, y, z, one], 0).astype(np.float32)
    bd = np.stack([x*x + y*y + z*z, x, y, z, one], 0).astype(np.float32)
    bmh, bml = split2(bm)
    bdh, bdl = split2(bd)
    basis = np.zeros((2, 30, VB), np.float32)
    basis[0] = np.concatenate([bmh, bml, bmh], 0)
    basis[1, 0:15] = np.concatenate([bdh, bdl, bdh], 0)
    # basis and chunk-0 W/U ship as one [2, 30, VB+128] tensor (one DMA pair)

    logop = np.log(np.maximum(op, 1e-30)).astype(np.float32)

    in_maps = []
    scatter = []  # per core: list of (chunk, slot, bx, by, bz)
    for c in range(N_CORES):
        wu = np.zeros((NCHUNK, 2, 30, 128), np.float32)
        ft = np.zeros((NCHUNK, 128, NBC * FCOL), np.float32)
        sc = []
        for k, bin_blocks in enumerate(packing[c]):
            p0 = 0
            for s, (bx, by, bz, center, sel) in enumerate(bin_blocks):
                K = len(sel)
                mu = (means3d[sel] - center).astype(np.float64)
                A = cinv[sel]
                Amu = np.einsum("kij,kj->ki", A, mu)
                cquad = np.einsum("ki,ki->k", Amu, mu)
                W = np.stack([
                    A[:, 0, 0], A[:, 1, 1], A[:, 2, 2],
                    2*A[:, 0, 1], 2*A[:, 0, 2], 2*A[:, 1, 2],
                    -2*Amu[:, 0], -2*Amu[:, 1], -2*Amu[:, 2],
                    cquad,
                ], 0) * (-0.5)
                W[9] += logop[sel]
                U = np.stack([
                    -np.ones(K), 2*mu[:, 0], 2*mu[:, 1], 2*mu[:, 2],
                    R2[sel] - (mu*mu).sum(1),
                ], 0).astype(np.float32)
                wh, wl = split2(W.astype(np.float32))
                uh, ul = split2(U)
                wu[k, 0, :, p0:p0+K] = np.concatenate([wh, wh, wl], 0)
                wu[k, 1, 0:15, p0:p0+K] = np.concatenate([uh, uh, ul], 0)
                ft[k, p0:p0+K, s*FCOL:s*FCOL+F] = features[sel]
                ft[k, p0:p0+K, s*FCOL+F] = 1.0
                sc.append((k, s, bx, by, bz))
                p0 += K
        bw = np.concatenate([basis, wu[0]], axis=2)  # [2, 30, VB+128]
        in_maps.append({"bw": bw.astype(ml_dtypes.bfloat16),
                        "wu": wu[1:].astype(ml_dtypes.bfloat16),
                        "ft": ft.astype(ml_dtypes.bfloat16)})
        scatter.append(sc)

    # Voxels whose sphere test is numerically ambiguous (|exact margin| below
    # the quadratic-form evaluation error) get recomputed exactly on the host.
    patch = []
    gf64 = g.reshape(-1, 3).astype(np.float64)
    live = np.where(op > 0)[0]
    d2 = ((means3d[live, None, :].astype(np.float64) - gf64[None, :, :]) ** 2).sum(-1)
    marg = np.abs(d2 - R2[live][:, None].astype(np.float64))
    risky = np.unique(np.where(marg < 5e-3)[1])
    for v in risky:
        dv, fv = 0.0, np.zeros(F, np.float64)
        for m in live:
            dd = means3d[m].astype(np.float64) - gf64[v]
            if (dd * dd).sum() < R2[m]:
                cm = op[m] * np.exp(-0.5 * dd @ cinv[m] @ dd)
                dv += cm
                fv += cm * features[m]
        patch.append((v, dv, fv / max(dv, 1e-6)))

    return cfg, in_maps, scatter, patch


def _build(cfg):
    """Build + compile the 8-core SPMD Tile kernel for a given (NCHUNK, NBC)."""
    if cfg in _BUILD_CACHE:
        return _BUILD_CACHE[cfg]
    NCHUNK, NBC = cfg
    FW = NBC * FCOL           # psum columns per voxel-slice group
    OW = NVC * NBC * (F + 1)  # output columns: feats then densities

    NH = NBC // 2             # blocks per half (feature matmuls split in halves
    FH = NH * FCOL            # so each chunk's accumulator fits 2 psum banks)
    assert NBC % 2 == 0 and NVC * FH <= 512

    nc = bacc.Bacc("TRN2", target_bir_lowering=False, debug=False,
                   enable_asserts=False, num_devices=N_CORES)
    dt = mybir.dt.float32
    bf = mybir.dt.bfloat16
    bw_ap = nc.dram_tensor("bw", [2, 30, VB + 128], bf, kind="ExternalInput").ap()
    wu_ap = nc.dram_tensor("wu", [NCHUNK - 1, 2, 30, 128], bf,
                           kind="ExternalInput").ap()
    ft_ap = nc.dram_tensor("ft", [NCHUNK, 128, FW], bf, kind="ExternalInput").ap()
    out_ap = nc.dram_tensor("out", [NCHUNK, 128, OW], dt, kind="ExternalOutput").ap()

    with tile.TileContext(nc) as tc, ExitStack() as ctx:
        const = ctx.enter_context(tc.tile_pool(name="const", bufs=1))
        inp = ctx.enter_context(tc.tile_pool(name="inp", bufs=2))
        work = ctx.enter_context(tc.tile_pool(name="work", bufs=1))
        outp = ctx.enter_context(tc.tile_pool(name="outp", bufs=2))
        qps = ctx.enter_context(tc.tile_pool(name="qps", bufs=1, space="PSUM"))
        fps = ctx.enter_context(tc.tile_pool(name="fps", bufs=2, space="PSUM"))
        # psum budget: arg 2 banks + nd2 2 + acc 2x2 = 8 exactly

        # warm the ACT exp table (ACT_TABLE_LOAD ~1.3us) and the PE clock (HAM
        # un-throttles after ~3.4us of sustained matmul activity) while the
        # input DMAs are in flight
        warm = const.tile([1, 1], dt)
        nc.vector.memset(warm[:], 0.0)
        nc.scalar.activation(warm[:], warm[:], mybir.ActivationFunctionType.Exp)


        # basis + chunk-0 W/U arrive as one tensor (2 DMAs on sync)
        bw_t = const.tile([128, VB + 128], bf)
        nc.sync.dma_start(bw_t[0:30, :], bw_ap[0])
        nc.scalar.dma_start(bw_t[64:94, :], bw_ap[1])

        # remaining chunks' W/U on gpsimd (inputs drain early; only the kernel-
        # tail gpsimd drain is slow), features split scalar/sync (HWDGE)
        wu_lhs = [(bw_t[0:30, VB:VB+128], bw_t[64:79, VB:VB+128])]
        ft_ts = []
        for k in range(1, NCHUNK):
            wu_t = inp.tile([128, 128], bf, tag=f"wu{k}")
            nc.gpsimd.dma_start(wu_t[0:30, :], wu_ap[k - 1, 0])
            nc.gpsimd.dma_start(wu_t[64:94, :], wu_ap[k - 1, 1])
            wu_lhs.append((wu_t[0:30, :], wu_t[64:79, :]))
        for k in range(NCHUNK):
            ft_t = inp.tile([128, FW], bf, tag=f"ft{k}")
            (nc.sync if k % 2 else nc.scalar).dma_start(ft_t[:], ft_ap[k])
            ft_ts.append(ft_t)

        # quadratic forms for all chunks into strided psum (chunk k at col 512k)
        arg_ps = qps.tile([128, 512 * NCHUNK], dt, tag="arg")
        nd2_ps = qps.tile([128, 512 * NCHUNK], dt, tag="nd2")
        # PE pipeline warmup into an unused psum column (VB..511 is dead space)
        nc.tensor.matmul(arg_ps[0:1, 508:509], warm[:], warm[:])
        for k in range(NCHUNK):
            nc.tensor.matmul(arg_ps[:, 512*k:512*k+VB], wu_lhs[k][0],
                             bw_t[0:30, 0:VB], tile_position=(0, 0))
            nc.tensor.matmul(nd2_ps[:, 512*k:512*k+VB], wu_lhs[k][1],
                             bw_t[64:79, 0:VB], tile_position=(64, 0))

        # single exp + single mask-multiply over all chunks (strided views)
        argv = arg_ps[:].rearrange("p (c x) -> p c x", c=NCHUNK)[:, :, 0:VB]
        nd2v = nd2_ps[:].rearrange("p (c x) -> p c x", c=NCHUNK)[:, :, 0:VB]
        t_t = work.tile([128, NCHUNK * VB], dt, tag="texp")
        tv = t_t[:].rearrange("p (c x) -> p c x", c=NCHUNK)
        nc.scalar.activation(tv, argv, mybir.ActivationFunctionType.Exp)
        contrib = work.tile([128, NCHUNK * VB], bf, tag="contrib")
        cv = contrib[:].rearrange("p (c x) -> p c x", c=NCHUNK)
        nc.vector.scalar_tensor_tensor(
            cv, nd2v, 0.0, tv,
            op0=mybir.AluOpType.is_gt, op1=mybir.AluOpType.mult)

        for k in range(NCHUNK):
            # feature/density accumulation into [128, (h, j) x FH] (2 banks):
            # half h of the blocks at column 512h, voxel slice j at + FH*j
            f_ps = fps.tile([128, 1024], dt, tag="acc")
            for j in range(NVC):
                lhs = contrib[:, k*VB + j*128 : k*VB + (j+1)*128]
                for h in range(2):
                    nc.tensor.matmul(f_ps[:, 512*h + FH*j : 512*h + FH*(j+1)],
                                     lhs, ft_ts[k][:, FH*h : FH*(h+1)])

            # normalization (per-partition = per-voxel)
            acc = f_ps[:].rearrange("p (h x) -> p h x", h=2)[:, :, 0:NVC*FH] \
                         .rearrange("p h (j b c) -> p h j b c", j=NVC, b=NH)
            dens = acc[:, :, :, :, F]         # [128, 2, 3, NH] raw density
            dmax = work.tile([128, 2 * NVC * NH], dt, tag=f"dmax{k}")
            nc.vector.tensor_scalar_max(dmax[:], dens, 1e-6)
            rec = work.tile([128, 2 * NVC * NH], dt, tag=f"rec{k}")
            nc.vector.reciprocal(rec[:], dmax[:])
            rec4 = rec[:].rearrange("p (h j b) -> p h j b", h=2, j=NVC)

            out_t = outp.tile([128, OW], dt, tag="out")
            nf = NVC * NH * F
            of5 = out_t[:, 0:2*nf] \
                .rearrange("p (h j b c) -> p h j b c", h=2, j=NVC, b=NH)
            nc.vector.tensor_mul(
                of5, acc[:, :, :, :, 0:F],
                rec4.unsqueeze(4).broadcast_to((128, 2, NVC, NH, F)))
            od3 = out_t[:, 2*nf:OW].rearrange("p (h j b) -> p h j b", h=2, j=NVC)
            nc.scalar.copy(od3[:], dens)

            (nc.sync if k % 2 else nc.scalar).dma_start(out_ap[k], out_t[:])

    nc.compile()
    _BUILD_CACHE[cfg] = nc
    return nc


def _scatter(cfg, outs, scatter):
    """Map per-core device outputs back onto the full grid."""
    NCHUNK, NBC = cfg
    NH = NBC // 2
    nf = NVC * NH * F
    dens = np.zeros(GS, np.float32)
    feats = np.zeros((*GS, F), np.float32)
    for c in range(N_CORES):
        o = outs[c]  # [NCHUNK, 128, OW]
        fe = o[:, :, 0:2*nf].reshape(-1, 128, 2, NVC, NH, F)
        de = o[:, :, 2*nf:].reshape(-1, 128, 2, NVC, NH)
        for (k, s, bx, by, bz) in scatter[c]:
            h, b = divmod(s, NH)
            fb = fe[k, :, h, :, b, :].transpose(1, 0, 2).reshape(BX, BY, BZ, F)
            db = de[k, :, h, :, b].transpose(1, 0).reshape(BX, BY, BZ)
            feats[bx*BX:(bx+1)*BX, by*BY:(by+1)*BY, bz*BZ:(bz+1)*BZ] = fb
            dens[bx*BX:(bx+1)*BX, by*BY:(by+1)*BY, bz*BZ:(bz+1)*BZ] = db
    return dens[..., None], feats


def kernel(means3d, opacities, covariances, features):
    global LAST_RESULTS
    means3d = np.asarray(means3d, np.float32)
    opacities = np.asarray(opacities, np.float32)
    covariances = np.asarray(covariances, np.float32)
    features = np.asarray(features, np.float32)

    cfg, in_maps, scatter, patch = _prepare(means3d, opacities, covariances,
                                            features)
    nc = _build(cfg)

    res = bass_utils.run_bass_kernel_spmd(nc, in_maps, core_ids=list(range(N_CORES)))
    LAST_RESULTS = res

    dens, feats = _scatter(cfg, [res.results[c]["out"] for c in range(N_CORES)],
                           scatter)
    df, ff = dens.reshape(-1), feats.reshape(-1, F)
    for v, dv, fv in patch:
        df[v] = dv
        ff[v] = fv
    return dens, feats


# revision 39
# speedup vs baseline: 1.2108x; 1.2108x over previous
"""Trainium2 Bass kernel for the MemoryEfficientVoxelizer problem.

Splats N=512 3D Gaussians onto an (80, 80, 12) voxel grid:
    contrib[n, v] = opac[n] * exp(-0.5 * (g_v - mu_n)^T Cinv_n (g_v - mu_n))
                    * [ |g_v - mu_n|^2 < (3 * sigma_max_n)^2 ]
    density[v]    = sum_n contrib[n, v]
    feats[v, :]   = (sum_n contrib[n, v] * features[n, :]) / max(density[v], 1e-6)

Strategy (8 NeuronCores, voxel-grid sharded):
  - Partition the grid into 4x8x12-voxel blocks (200 blocks, 384 voxels each).
  - On host: cull Gaussians per block (point-to-bbox distance vs the 3*sigma_max
    sphere) giving a sparse (block, gaussian) pair list; balance pairs across
    cores; bin-pack each core's blocks into 128-pair chunks.
  - Both the Mahalanobis form and the sphere test are quadratic polynomials in
    the voxel coordinates, so per chunk the device computes them as tiny
    matmuls against a *shared* block-local polynomial basis:
        maha_arg[pair, v] = W[pair, :10] @ B10[:, v]   (K=10, fp32)
        negd2c[pair, v]   = U[pair, :5]  @ B5[:, v]    (K=5,  fp32, row group 2)
    with -0.5 and log(opacity) folded into W on the host.
  - contrib = (negd2c > 0) * exp(maha_arg): one ScalarE Exp + one fused
    VectorE scalar_tensor_tensor.
  - Feature/density accumulation per 128-voxel slice: contrib slice (stationary)
    x zero-masked per-block feature columns (features + ones column), giving
    [voxel, 33] tiles; normalization is then per-partition native.
"""

import os
import numpy as np
import ml_dtypes
from contextlib import ExitStack

import concourse.bass as bass
import concourse.tile as tile
from concourse import bacc, mybir
from concourse import bass_utils

# ---- problem constants (fixed by the reference) ----
VOL_MIN = np.array([-20.0, -20.0, -2.0], np.float32)
VOL_MAX = np.array([20.0, 20.0, 4.4], np.float32)
VOX = 0.5
GS = (80, 80, 12)
F = 32
OPACITY_THRESH = 1e-4
N_CORES = 8

# block shape in voxels
BX, BY, BZ = 4, 8, 12
VB = BX * BY * BZ          # 384 voxels / block
NVC = VB // 128            # 3 voxel slices of 128
FCOL = F + 1               # features + ones column

LAST_RESULTS = None        # BassKernelResults of the last hardware run
_BUILD_CACHE = {}


def _grid_coords():
    coords = [np.arange(s, dtype=np.float32) + 0.5 for s in GS]
    g = np.stack(np.meshgrid(*coords, indexing="ij"), axis=-1)
    return g * VOX + VOL_MIN  # [80,80,12,3] voxel centers


def _prepare(means3d, opacities, covariances, features):
    """Host-side culling, packing and weight computation.

    Returns (cfg, in_maps, scatter) where scatter maps device output slots back
    to grid blocks.
    """
    g = _grid_coords()
    sigma2 = np.diagonal(covariances, axis1=1, axis2=2).astype(np.float64)
    sigma = np.sqrt(sigma2).astype(np.float32)
    op = opacities[:, 0].astype(np.float32).copy()
    keep = ((means3d + 3 * sigma) > VOL_MIN).all(1) & \
           ((means3d - 3 * sigma) < VOL_MAX).all(1) & (op > OPACITY_THRESH)
    op = op * keep
    cinv = np.linalg.inv(covariances.astype(np.float64))
    smax = sigma.max(1)
    R2 = ((3.0 * smax) ** 2).astype(np.float32)  # squared sphere radii

    nbx, nby, nbz = GS[0] // BX, GS[1] // BY, GS[2] // BZ
    blocks = []  # (bx, by, bz, center[3], sel_indices)
    live = np.where(op > 0)[0]
    for bx in range(nbx):
        for by in range(nby):
            for bz in range(nbz):
                vox = g[bx*BX:(bx+1)*BX, by*BY:(by+1)*BY, bz*BZ:(bz+1)*BZ]
                lo = vox.reshape(-1, 3).min(0)
                hi = vox.reshape(-1, 3).max(0)
                center = (lo + hi) * 0.5
                d = np.maximum(np.maximum(lo - means3d[live], means3d[live] - hi), 0.0)
                sel = live[(d * d).sum(1) < R2[live]]
                if len(sel):
                    blocks.append((bx, by, bz, center, sel))

    # greedy balance blocks across cores by pair count
    blocks.sort(key=lambda b: -len(b[4]))
    core_blocks = [[] for _ in range(N_CORES)]
    core_pairs = [0] * N_CORES
    for b in blocks:
        c = min(range(N_CORES), key=lambda i: (core_pairs[i], len(core_blocks[i])))
        core_blocks[c].append(b)
        core_pairs[c] += len(b[4])

    # choose (NCHUNK, NBC) and first-fit-decreasing bin packing per core
    def pack(blks, nchunk, nbc):
        bins = [[] for _ in range(nchunk)]
        cnt = [0] * nchunk
        for b in sorted(blks, key=lambda b: -len(b[4])):
            placed = False
            for i in range(nchunk):
                if cnt[i] + len(b[4]) <= 128 and len(bins[i]) < nbc:
                    bins[i].append(b)
                    cnt[i] += len(b[4])
                    placed = True
                    break
            if not placed:
                return None
        return bins

    packing = None
    for nchunk in range(1, 16):
        maxb = max((len(cb) for cb in core_blocks), default=1)
        for nbc in range(max(2, -(-maxb // nchunk)), 16):
            if nbc % 2:
                continue
            if (nbc // 2) * FCOL * NVC > 512:
                break
            trial = [pack(cb, nchunk, nbc) for cb in core_blocks]
            if all(t is not None for t in trial):
                packing = trial
                NCHUNK, NBC = nchunk, nbc
                break
        if packing is not None:
            break
    assert packing is not None, "bin packing failed (unexpected input distribution)"

    cfg = (NCHUNK, NBC)

    def split2(a):  # bf16 Kahan pair: a ~= hi + lo
        hi = a.astype(ml_dtypes.bfloat16).astype(np.float32)
        lo = (a - hi).astype(ml_dtypes.bfloat16).astype(np.float32)
        return hi, lo

    # Shared block-local polynomial basis. Quadratic forms run as single bf16
    # matmuls via a Kahan split: stat [Xh; Xh; Xl] against basis [Bh; Bl; Bh]
    # computes XhBh + XhBl + XlBh in one K-deep contraction (bf16 products are
    # exact in fp32).
    #   group 0 (-> partitions 0..29):  3x10 rows, Mahalanobis basis
    #   group 1 (-> partitions 64..78): 3x5 rows, sphere-test basis
    loc = g[:BX, :BY, :BZ].reshape(-1, 3)
    loc = loc - (loc.min(0) + loc.max(0)) * 0.5  # block-local coords (same for all blocks)
    x, y, z = loc[:, 0], loc[:, 1], loc[:, 2]
    one = np.ones_like(x)
    bm = np.stack([x*x, y*y, z*z, x*y, x*z, y*z, x, y, z, one], 0).astype(np.float32)
    bd = np.stack([x*x + y*y + z*z, x, y, z, one], 0).astype(np.float32)
    bmh, bml = split2(bm)
    bdh, bdl = split2(bd)
    basis = np.zeros((2, 30, VB), np.float32)
    basis[0] = np.concatenate([bmh, bml, bmh], 0)
    basis[1, 0:15] = np.concatenate([bdh, bdl, bdh], 0)
    # basis and chunk-0 W/U ship as one [2, 30, VB+128] tensor (one DMA pair)

    logop = np.log(np.maximum(op, 1e-30)).astype(np.float32)

    in_maps = []
    scatter = []  # per core: list of (chunk, slot, bx, by, bz)
    for c in range(N_CORES):
        wu = np.zeros((NCHUNK, 2, 30, 128), np.float32)
        ft = np.zeros((NCHUNK, 128, NBC * FCOL), np.float32)
        sc = []
        for k, bin_blocks in enumerate(packing[c]):
            p0 = 0
            for s, (bx, by, bz, center, sel) in enumerate(bin_blocks):
                K = len(sel)
                mu = (means3d[sel] - center).astype(np.float64)
                A = cinv[sel]
                Amu = np.einsum("kij,kj->ki", A, mu)
                cquad = np.einsum("ki,ki->k", Amu, mu)
                W = np.stack([
                    A[:, 0, 0], A[:, 1, 1], A[:, 2, 2],
                    2*A[:, 0, 1], 2*A[:, 0, 2], 2*A[:, 1, 2],
                    -2*Amu[:, 0], -2*Amu[:, 1], -2*Amu[:, 2],
                    cquad,
                ], 0) * (-0.5)
                W[9] += logop[sel]
                U = np.stack([
                    -np.ones(K), 2*mu[:, 0], 2*mu[:, 1], 2*mu[:, 2],
                    R2[sel] - (mu*mu).sum(1),
                ], 0).astype(np.float32)
                wh, wl = split2(W.astype(np.float32))
                uh, ul = split2(U)
                wu[k, 0, :, p0:p0+K] = np.concatenate([wh, wh, wl], 0)
                wu[k, 1, 0:15, p0:p0+K] = np.concatenate([uh, uh, ul], 0)
                ft[k, p0:p0+K, s*FCOL:s*FCOL+F] = features[sel]
                ft[k, p0:p0+K, s*FCOL+F] = 1.0
                sc.append((k, s, bx, by, bz))
                p0 += K
        bw = np.concatenate([basis, wu[0]], axis=2)  # [2, 30, VB+128]
        in_maps.append({"bw": bw.astype(ml_dtypes.bfloat16),
                        "wu": wu[1:].astype(ml_dtypes.bfloat16),
                        "ft": ft.astype(ml_dtypes.bfloat16)})
        scatter.append(sc)

    # Voxels whose sphere test is numerically ambiguous (|exact margin| below
    # the quadratic-form evaluation error) get recomputed exactly on the host.
    patch = []
    gf64 = g.reshape(-1, 3).astype(np.float64)
    live = np.where(op > 0)[0]
    d2 = ((means3d[live, None, :].astype(np.float64) - gf64[None, :, :]) ** 2).sum(-1)
    marg = np.abs(d2 - R2[live][:, None].astype(np.float64))
    risky = np.unique(np.where(marg < 5e-3)[1])
    for v in risky:
        dv, fv = 0.0, np.zeros(F, np.float64)
        for m in live:
            dd = means3d[m].astype(np.float64) - gf64[v]
            if (dd * dd).sum() < R2[m]:
                cm = op[m] * np.exp(-0.5 * dd @ cinv[m] @ dd)
                dv += cm
                fv += cm * features[m]
        patch.append((v, dv, fv / max(dv, 1e-6)))

    return cfg, in_maps, scatter, patch


def _build(cfg):
    """Build + compile the 8-core SPMD Tile kernel for a given (NCHUNK, NBC)."""
    if cfg in _BUILD_CACHE:
        return _BUILD_CACHE[cfg]
    NCHUNK, NBC = cfg
    FW = NBC * FCOL           # psum columns per voxel-slice group
    OW = NVC * NBC * (F + 1)  # output columns: feats then densities

    NH = NBC // 2             # blocks per half (feature matmuls split in halves
    FH = NH * FCOL            # so each chunk's accumulator fits 2 psum banks)
    assert NBC % 2 == 0 and NVC * FH <= 512

    nc = bacc.Bacc("TRN2", target_bir_lowering=False, debug=False,
                   enable_asserts=False, num_devices=N_CORES)
    dt = mybir.dt.float32
    bf = mybir.dt.bfloat16
    bw_ap = nc.dram_tensor("bw", [2, 30, VB + 128], bf, kind="ExternalInput").ap()
    wu_ap = nc.dram_tensor("wu", [NCHUNK - 1, 2, 30, 128], bf,
                           kind="ExternalInput").ap()
    ft_ap = nc.dram_tensor("ft", [NCHUNK, 128, FW], bf, kind="ExternalInput").ap()
    out_ap = nc.dram_tensor("out", [NCHUNK, 128, OW], dt, kind="ExternalOutput").ap()

    with tile.TileContext(nc) as tc, ExitStack() as ctx:
        const = ctx.enter_context(tc.tile_pool(name="const", bufs=1))
        inp = ctx.enter_context(tc.tile_pool(name="inp", bufs=2))
        work = ctx.enter_context(tc.tile_pool(name="work", bufs=1))
        outp = ctx.enter_context(tc.tile_pool(name="outp", bufs=2))
        qps = ctx.enter_context(tc.tile_pool(name="qps", bufs=1, space="PSUM"))
        fps = ctx.enter_context(tc.tile_pool(name="fps", bufs=2, space="PSUM"))
        # psum budget: arg 2 banks + nd2 2 + acc 2x2 = 8 exactly

        # warm the ACT exp table (ACT_TABLE_LOAD ~1.3us) and the PE clock (HAM
        # un-throttles after ~3.4us of sustained matmul activity) while the
        # input DMAs are in flight
        warm = const.tile([1, 1], dt)
        nc.vector.memset(warm[:], 0.0)
        nc.scalar.activation(warm[:], warm[:], mybir.ActivationFunctionType.Exp)


        # basis + chunk-0 W/U arrive as one tensor (2 DMAs on sync)
        bw_t = const.tile([128, VB + 128], bf)
        nc.sync.dma_start(bw_t[0:30, :], bw_ap[0])
        nc.scalar.dma_start(bw_t[64:94, :], bw_ap[1])

        # remaining chunks' W/U on gpsimd (inputs drain early; only the kernel-
        # tail gpsimd drain is slow), features split scalar/sync (HWDGE)
        wu_lhs = [(bw_t[0:30, VB:VB+128], bw_t[64:79, VB:VB+128])]
        ft_ts = []
        for k in range(1, NCHUNK):
            wu_t = inp.tile([128, 128], bf, tag=f"wu{k}")
            nc.gpsimd.dma_start(wu_t[0:30, :], wu_ap[k - 1, 0])
            nc.gpsimd.dma_start(wu_t[64:94, :], wu_ap[k - 1, 1])
            wu_lhs.append((wu_t[0:30, :], wu_t[64:79, :]))
        for k in range(NCHUNK):
            ft_t = inp.tile([128, FW], bf, tag=f"ft{k}")
            (nc.sync if k % 2 else nc.scalar).dma_start(ft_t[:], ft_ap[k])
            ft_ts.append(ft_t)

        # quadratic forms for all chunks into strided psum (chunk k at col 512k)
        arg_ps = qps.tile([128, 512 * NCHUNK], dt, tag="arg")
        nd2_ps = qps.tile([128, 512 * NCHUNK], dt, tag="nd2")
        # PE pipeline warmup into an unused psum column (VB..511 is dead space)
        nc.tensor.matmul(arg_ps[0:1, 508:509], warm[:], warm[:])
        for k in range(NCHUNK):
            nc.tensor.matmul(arg_ps[:, 512*k:512*k+VB], wu_lhs[k][0],
                             bw_t[0:30, 0:VB], tile_position=(0, 0))
            nc.tensor.matmul(nd2_ps[:, 512*k:512*k+VB], wu_lhs[k][1],
                             bw_t[64:79, 0:VB], tile_position=(64, 0))

        # per-chunk exp + fused mask-multiply so chunk 0's downstream chain
        # (feature matmuls + normalization on DVE) starts as early as possible
        contrib = work.tile([128, NCHUNK * VB], bf, tag="contrib")
        for k in range(NCHUNK):
            t_t = work.tile([128, VB], dt, tag=f"texp{k}")
            nc.scalar.activation(t_t[:], arg_ps[:, 512*k:512*k+VB],
                                 mybir.ActivationFunctionType.Exp)
            nc.vector.scalar_tensor_tensor(
                contrib[:, k*VB:(k+1)*VB], nd2_ps[:, 512*k:512*k+VB], 0.0,
                t_t[:], op0=mybir.AluOpType.is_gt, op1=mybir.AluOpType.mult)

        for k in range(NCHUNK):
            # feature/density accumulation into [128, (h, j) x FH] (2 banks):
            # half h of the blocks at column 512h, voxel slice j at + FH*j
            f_ps = fps.tile([128, 1024], dt, tag="acc")
            for j in range(NVC):
                lhs = contrib[:, k*VB + j*128 : k*VB + (j+1)*128]
                for h in range(2):
                    nc.tensor.matmul(f_ps[:, 512*h + FH*j : 512*h + FH*(j+1)],
                                     lhs, ft_ts[k][:, FH*h : FH*(h+1)])

            # normalization (per-partition = per-voxel)
            acc = f_ps[:].rearrange("p (h x) -> p h x", h=2)[:, :, 0:NVC*FH] \
                         .rearrange("p h (j b c) -> p h j b c", j=NVC, b=NH)
            dens = acc[:, :, :, :, F]         # [128, 2, 3, NH] raw density
            dmax = work.tile([128, 2 * NVC * NH], dt, tag=f"dmax{k}")
            nc.vector.tensor_scalar_max(dmax[:], dens, 1e-6)
            rec = work.tile([128, 2 * NVC * NH], dt, tag=f"rec{k}")
            nc.vector.reciprocal(rec[:], dmax[:])
            rec4 = rec[:].rearrange("p (h j b) -> p h j b", h=2, j=NVC)

            out_t = outp.tile([128, OW], dt, tag="out")
            nf = NVC * NH * F
            of5 = out_t[:, 0:2*nf] \
                .rearrange("p (h j b c) -> p h j b c", h=2, j=NVC, b=NH)
            nc.vector.tensor_mul(
                of5, acc[:, :, :, :, 0:F],
                rec4.unsqueeze(4).broadcast_to((128, 2, NVC, NH, F)))
            od3 = out_t[:, 2*nf:OW].rearrange("p (h j b) -> p h j b", h=2, j=NVC)
            nc.scalar.copy(od3[:], dens)

            (nc.sync if k % 2 else nc.scalar).dma_start(out_ap[k], out_t[:])

    nc.compile()
    _BUILD_CACHE[cfg] = nc
    return nc


def _scatter(cfg, outs, scatter):
    """Map per-core device outputs back onto the full grid."""
    NCHUNK, NBC = cfg
    NH = NBC // 2
    nf = NVC * NH * F
    dens = np.zeros(GS, np.float32)
    feats = np.zeros((*GS, F), np.float32)
    for c in range(N_CORES):
        o = outs[c]  # [NCHUNK, 128, OW]
        fe = o[:, :, 0:2*nf].reshape(-1, 128, 2, NVC, NH, F)
        de = o[:, :, 2*nf:].reshape(-1, 128, 2, NVC, NH)
        for (k, s, bx, by, bz) in scatter[c]:
            h, b = divmod(s, NH)
            fb = fe[k, :, h, :, b, :].transpose(1, 0, 2).reshape(BX, BY, BZ, F)
            db = de[k, :, h, :, b].transpose(1, 0).reshape(BX, BY, BZ)
            feats[bx*BX:(bx+1)*BX, by*BY:(by+1)*BY, bz*BZ:(bz+1)*BZ] = fb
            dens[bx*BX:(bx+1)*BX, by*BY:(by+1)*BY, bz*BZ:(bz+1)*BZ] = db
    return dens[..., None], feats


def kernel(means3d, opacities, covariances, features):
    global LAST_RESULTS
    means3d = np.asarray(means3d, np.float32)
    opacities = np.asarray(opacities, np.float32)
    covariances = np.asarray(covariances, np.float32)
    features = np.asarray(features, np.float32)

    cfg, in_maps, scatter, patch = _prepare(means3d, opacities, covariances,
                                            features)
    nc = _build(cfg)

    res = bass_utils.run_bass_kernel_spmd(nc, in_maps, core_ids=list(range(N_CORES)))
    LAST_RESULTS = res

    dens, feats = _scatter(cfg, [res.results[c]["out"] for c in range(N_CORES)],
                           scatter)
    df, ff = dens.reshape(-1), feats.reshape(-1, F)
    for v, dv, fv in patch:
        df[v] = dv
        ff[v] = fv
    return dens, feats
